# revision 8
# baseline (speedup 1.0000x reference)
"""Trainium2 Bass kernel for the sparse_attention nn problem.

Math (per sample n, all shapes per sample):
  t1_d = x shifted by 2*(d-1) rows (zero pad), d in {0,1,2}
  t2_d = p2w_d * t1_d
  t3_d = x shifted by 2*(d-1) cols (zero pad)
  t4   = roll(x, 1, rows);  t6 = t4 + x
  t7[k=(d,c1), c] = sum_p t2_d[c1,p] * t6[c,p] / 56
  t8full = W'' @ t4 with W''[c,c'] = conv_w[c%4,c'] * p5w[c']   (t9 = t8full*x)
  t10[c,p] = sum_{d,c1} t7[(d,c1),c] * t3_d[c1,p] / sqrt(384)
  out = t9 + t10
k-order is permuted (d-major) consistently in t7/t10 -> result unchanged.
Scales folded into p2w on host: p2w' = p2w / (56*sqrt(384)).

Sharding: pure data parallel over batch (4 samples per core, 8 cores).

Layout trick: contraction over p=3136 needs p on partitions. x is transposed
on the PE (28 transposes of (128x112) per sample) into PSUM in two half
groups; t2^T and t6^T are then built by DVE directly from PSUM (bf16 2-byte
operands keep the 2x DVE mode even from PSUM).
"""

import math
import numpy as np
import ml_dtypes

N, C, H, W, G = 32, 128, 56, 56, 32
HW = H * W                # 3136
NCORES = 8
NS = N // NCORES          # 4 samples per core
PCH = 2 * W               # 112 = p-chunk (2 image rows)
NCH = HW // PCH           # 28 chunks
HCH = NCH // 2            # 14 chunks per psum half-group
GB = HCH + 1              # 15 blocks per psum group (one chunk overlap)
FCH = 448                 # t10/t8 free chunk
NF = HW // FCH            # 7
SCALE = 1.0 / (56.0 * math.sqrt(384.0))
BF16NP = ml_dtypes.bfloat16

_CACHE = {}


def _body(tc, bass, mybir, xs, p2wt, w2t, ident, out, BF, F32, ctx):
    nc = tc.nc
    mult = mybir.AluOpType.mult
    addop = mybir.AluOpType.add

    consts = ctx.enter_context(tc.tile_pool(name="consts", bufs=1))
    p_x32 = ctx.enter_context(tc.tile_pool(name="x32", bufs=2))
    p_xbf = ctx.enter_context(tc.tile_pool(name="xbf", bufs=2))
    p_t3 = ctx.enter_context(tc.tile_pool(name="t3", bufs=2))
    p_t2t = ctx.enter_context(tc.tile_pool(name="t2t", bufs=1))
    p_t6n = ctx.enter_context(tc.tile_pool(name="t6n", bufs=2))
    p_t6t = ctx.enter_context(tc.tile_pool(name="t6t", bufs=2))
    p_t7sb = ctx.enter_context(tc.tile_pool(name="t7sb", bufs=2))
    p_fsb = ctx.enter_context(tc.tile_pool(name="fsb", bufs=3))
    p_out = ctx.enter_context(tc.tile_pool(name="outp", bufs=2))
    ps_xt = ctx.enter_context(tc.tile_pool(name="psxt", bufs=2, space="PSUM"))
    ps_t7 = ctx.enter_context(tc.tile_pool(name="pst7", bufs=1, space="PSUM"))
    ps_t10 = ctx.enter_context(tc.tile_pool(name="pst10", bufs=2, space="PSUM"))
    ps_t8 = ctx.enter_context(tc.tile_pool(name="pst8", bufs=1, space="PSUM"))

    p2wt_t = consts.tile([PCH, NCH * 3 * C], BF)
    nc.sync.dma_start(p2wt_t[:], p2wt)
    w2t_t = consts.tile([C, C], BF)
    nc.sync.dma_start(w2t_t[:], w2t)
    id_t = consts.tile([C, C], BF)
    nc.sync.dma_start(id_t[:], ident)

    def r3(ap, k):
        return ap.rearrange("p (j k) -> p j k", k=k)

    # t2^T buffer is persistent; its two boundary blocks (j=0,d=0) and
    # (j=27,d=2) are zero forever (the shifted x there is zero padding).
    t2t = p_t2t.tile([PCH, NCH * 3 * C], BF, tag="t2t")
    nc.gpsimd.memset(t2t[:, 0:C], 0.0)
    nc.gpsimd.memset(t2t[:, (NCH * 3 - 1) * C : NCH * 3 * C], 0.0)
    t2r = t2t[:].rearrange("p (j x) -> p j x", x=3 * C)
    pwr = p2wt_t[:].rearrange("p (j x) -> p j x", x=3 * C)

    for s in range(NS):
        x32 = p_x32.tile([C, HW], F32, tag="x32")
        nc.sync.dma_start(x32[:], xs[s])

        xbf = p_xbf.tile([C, HW], BF, tag="xbf")
        nc.gpsimd.tensor_copy(xbf[:], x32[:])  # fp32 -> bf16 cast on Pool

        xbf3 = xbf[:].rearrange("c (h w) -> c h w", h=H)
        # t3_0 (cols shifted +2 with zero pad) on ACT, t3_2 on DVE
        t30 = p_t3.tile([C, H, W], BF, tag="t30")
        nc.gpsimd.memset(t30[:, :, 0:2], 0.0)
        nc.scalar.copy(t30[:, :, 2:W], xbf3[:, :, 0 : W - 2])
        t32 = p_t3.tile([C, H, W], BF, tag="t32")
        nc.gpsimd.memset(t32[:, :, W - 2 : W], 0.0)
        nc.vector.tensor_copy(t32[:, :, 0 : W - 2], xbf3[:, :, 2:W])
        t30f = t30[:].rearrange("c h w -> c (h w)")
        t32f = t32[:].rearrange("c h w -> c (h w)")

        # --- transpose x into PSUM, two half-groups of 15 chunks ---
        # group 0 holds real chunks 0..14 at locals 0..14
        # group 1 holds real chunks 13..27 at locals 0..14
        xt0 = ps_xt.tile([PCH, GB * C], BF, tag="xt")
        xt1 = ps_xt.tile([PCH, GB * C], BF, tag="xt")
        for r in range(0, 15):
            nc.tensor.transpose(
                xt0[:, r * C : (r + 1) * C],
                xbf[:, r * PCH : (r + 1) * PCH],
                id_t[:],
            )
        for r in range(13, 28):
            nc.tensor.transpose(
                xt1[:, (r - 13) * C : (r - 12) * C],
                xbf[:, r * PCH : (r + 1) * PCH],
                id_t[:],
            )

        # --- t2^T = p2w^T * x^T(shifted) ---  (112 x 28*384, free = [j][d][c1])
        # output block (j, d) multiplies x^T real chunk (j + d - 1)
        for d in range(3):
            j0, j1 = max(0, 1 - d), min(HCH, HCH)  # g0 j range
            lo = j0 + d - 1
            nc.vector.tensor_tensor(
                out=t2r[:, j0:HCH, d * C : (d + 1) * C],
                in0=pwr[:, j0:HCH, d * C : (d + 1) * C],
                in1=r3(xt0[:, lo * C : (lo + HCH - j0) * C], C),
                op=mult,
            )
        for d in range(3):
            j0, j1 = HCH, NCH if d < 2 else NCH - 1  # g1 j range
            lo = j0 + d - 1 - 13
            nc.vector.tensor_tensor(
                out=t2r[:, j0:j1, d * C : (d + 1) * C],
                in0=pwr[:, j0:j1, d * C : (d + 1) * C],
                in1=r3(xt1[:, lo * C : (lo + j1 - j0) * C], C),
                op=mult,
            )

        # --- t6 = x + roll(x, 1 row) built in natural layout (free shifts) ---
        t6n = p_t6n.tile([C, HW], BF, tag="t6n")
        nc.vector.tensor_tensor(
            out=t6n[:, W:HW], in0=xbf[:, W:HW], in1=xbf[:, 0 : HW - W], op=addop
        )
        nc.vector.tensor_tensor(
            out=t6n[:, 0:W], in0=xbf[:, 0:W], in1=xbf[:, HW - W : HW], op=addop
        )
        # transpose t6n -> psum (two half groups), then batch-copy to SBUF
        t6t = p_t6t.tile([PCH, NCH * C], BF, tag="t6t")
        for g in range(2):
            yg = ps_xt.tile([PCH, GB * C], BF, tag="xt")
            for r in range(HCH):
                j = g * HCH + r
                nc.tensor.transpose(
                    yg[:, r * C : (r + 1) * C],
                    t6n[:, j * PCH : (j + 1) * PCH],
                    id_t[:],
                )
            nc.vector.tensor_copy(
                t6t[:, g * HCH * C : (g + 1) * HCH * C], yg[:, 0 : HCH * C]
            )

        # --- t7^T accumulation: (c x (d,c1)) ---
        t7ps = ps_t7.tile([C, 3 * C], F32, tag="t7")
        for j in range(NCH):
            nc.tensor.matmul(
                t7ps[:],
                t6t[:, j * C : (j + 1) * C],
                t2t[:, j * 3 * C : (j + 1) * 3 * C],
                start=(j == 0),
                stop=(j == NCH - 1),
            )
        t7T_sb = p_t7sb.tile([C, 3 * C], BF, tag="t7T")
        nc.vector.tensor_copy(t7T_sb[:], t7ps[:])
        t7dps = ps_t7.tile([C, 3 * C], BF, tag="t7")
        for d in range(3):
            nc.tensor.transpose(
                t7dps[:, d * C : (d + 1) * C],
                t7T_sb[:, d * C : (d + 1) * C],
                id_t[:],
            )
        t7d_sb = p_t7sb.tile([C, 3 * C], BF, tag="t7d")
        nc.vector.tensor_copy(t7d_sb[:], t7dps[:])

        # --- output stage ---
        outsb = p_out.tile([C, HW], F32, tag="outp")
        t3list = (t30f, xbf[:], t32f)
        for f in range(NF):
            lo = f * FCH
            t8ps = ps_t8.tile([C, FCH], F32, tag="t8")
            if f == 0:
                nc.tensor.matmul(
                    t8ps[:, 0:W], w2t_t[:], xbf[:, HW - W : HW],
                    start=True, stop=False,
                )
                nc.tensor.matmul(
                    t8ps[:, W:FCH], w2t_t[:], xbf[:, 0 : FCH - W],
                    start=False, stop=True,
                )
            else:
                nc.tensor.matmul(
                    t8ps[:], w2t_t[:], xbf[:, lo - W : lo + FCH - W],
                    start=True, stop=True,
                )
            t10ps = ps_t10.tile([C, FCH], F32, tag="t10")
            for d in range(3):
                nc.tensor.matmul(
                    t10ps[:],
                    t7d_sb[:, d * C : (d + 1) * C],
                    t3list[d][:, lo : lo + FCH],
                    start=(d == 0),
                    stop=(d == 2),
                )
            t8sb = p_fsb.tile([C, FCH], F32, tag="t8sb")
            nc.scalar.copy(t8sb[:], t8ps[:])
            t10sb = p_fsb.tile([C, FCH], F32, tag="t10sb")
            nc.scalar.copy(t10sb[:], t10ps[:])
            t9sb = p_fsb.tile([C, FCH], F32, tag="t9sb")
            nc.vector.tensor_tensor(
                out=t9sb[:], in0=t8sb[:], in1=x32[:, lo : lo + FCH], op=mult
            )
            nc.vector.tensor_tensor(
                out=outsb[:, lo : lo + FCH], in0=t9sb[:], in1=t10sb[:], op=addop
            )
        nc.sync.dma_start(out[s], outsb[:])


def build():
    if "nc" in _CACHE:
        return _CACHE["nc"]
    from contextlib import ExitStack

    import concourse.bass as bass
    import concourse.tile as tile
    from concourse import bacc, mybir

    BF = mybir.dt.bfloat16
    F32 = mybir.dt.float32
    nc = bacc.Bacc("TRN2", target_bir_lowering=False, debug=False)
    xs = nc.dram_tensor("xs", [NS, C, HW], F32, kind="ExternalInput").ap()
    p2wt = nc.dram_tensor("p2wt", [PCH, NCH * 3 * C], BF, kind="ExternalInput").ap()
    w2t = nc.dram_tensor("w2t", [C, C], BF, kind="ExternalInput").ap()
    ident = nc.dram_tensor("ident", [C, C], BF, kind="ExternalInput").ap()
    out = nc.dram_tensor("out", [NS, C, HW], F32, kind="ExternalOutput").ap()

    with tile.TileContext(nc) as tc:
        with __import__("contextlib").ExitStack() as ctx:
            _body(tc, bass, mybir, xs, p2wt, w2t, ident, out, BF, F32, ctx)
    nc.compile()
    _CACHE["nc"] = nc
    return nc


def host_inputs(x, p2w, p5w, conv_w):
    """Shard + prep per-core input maps from full inputs."""
    x = np.ascontiguousarray(np.asarray(x, dtype=np.float32).reshape(N, C, HW))
    p2w_ = (np.asarray(p2w, dtype=np.float32)[0] * SCALE).reshape(C, 3, HW)
    a = p2w_.transpose(2, 1, 0)                      # (p, d, c1)
    a = a.reshape(NCH, PCH, 3, C)                    # (j, pl, d, c1)
    a = np.ascontiguousarray(a.transpose(1, 0, 2, 3)).reshape(PCH, NCH * 3 * C)
    p2wt = a.astype(BF16NP)
    p5 = np.asarray(p5w, dtype=np.float32).reshape(C)
    cw = np.asarray(conv_w, dtype=np.float32)        # (C//G, C)
    W2 = cw[np.arange(C) % (C // G)] * p5[None, :]   # (c, c')
    w2t = np.ascontiguousarray(W2.T).astype(BF16NP)  # (c', c)
    ident = np.eye(C, dtype=BF16NP)
    in_maps = [
        {
            "xs": np.ascontiguousarray(x[i * NS : (i + 1) * NS]),
            "p2wt": p2wt,
            "w2t": w2t,
            "ident": ident,
        }
        for i in range(NCORES)
    ]
    return in_maps


def run(in_maps, trace=False, **kw):
    from concourse.bass_utils import run_bass_kernel_spmd

    nc = build()
    return run_bass_kernel_spmd(nc, in_maps, list(range(NCORES)), trace=trace, **kw)


def kernel(x, p2w, p5w, conv_w):
    in_maps = host_inputs(x, p2w, p5w, conv_w)
    res = run(in_maps)
    outs = [np.asarray(res.results[i]["out"]) for i in range(NCORES)]
    return np.concatenate(outs, axis=0).reshape(N, C, H, W).astype(np.float32)


# revision 9
# speedup vs baseline: 760.9769x; 760.9769x over previous
"""Trainium2 Bass kernel for the sparse_attention nn problem.

Math (per sample n, all shapes per sample):
  t1_d = x shifted by 2*(d-1) rows (zero pad), d in {0,1,2}
  t2_d = p2w_d * t1_d
  t3_d = x shifted by 2*(d-1) cols (zero pad)
  t4   = roll(x, 1, rows);  t6 = t4 + x
  t7[k=(d,c1), c] = sum_p t2_d[c1,p] * t6[c,p] / 56
  t8full = W'' @ t4 with W''[c,c'] = conv_w[c%4,c'] * p5w[c']   (t9 = t8full*x)
  t10[c,p] = sum_{d,c1} t7[(d,c1),c] * t3_d[c1,p] / sqrt(384)
  out = t9 + t10
k-order is permuted (d-major) consistently in t7/t10 -> result unchanged.
Scales folded into p2w on host: p2w' = p2w / (56*sqrt(384)).

Sharding: pure data parallel over batch (4 samples per core, 8 cores).

Layout trick: contraction over p=3136 needs p on partitions. x is transposed
on the PE (28 transposes of (128x112) per sample) into PSUM in two half
groups; t2^T and t6^T are then built by DVE directly from PSUM (bf16 2-byte
operands keep the 2x DVE mode even from PSUM).
"""

import math
import numpy as np
import ml_dtypes

N, C, H, W, G = 32, 128, 56, 56, 32
HW = H * W                # 3136
NCORES = 8
NS = N // NCORES          # 4 samples per core
PCH = 2 * W               # 112 = p-chunk (2 image rows)
NCH = HW // PCH           # 28 chunks
HCH = NCH // 2            # 14 chunks per psum half-group
GB = HCH + 1              # 15 blocks per psum group (one chunk overlap)
FCH = 448                 # t10/t8 free chunk
NF = HW // FCH            # 7
SCALE = 1.0 / (56.0 * math.sqrt(384.0))
BF16NP = ml_dtypes.bfloat16

_CACHE = {}


def _body(tc, bass, mybir, xs, p2wt, w2t, ident, out, BF, F32, ctx):
    nc = tc.nc
    mult = mybir.AluOpType.mult
    addop = mybir.AluOpType.add

    consts = ctx.enter_context(tc.tile_pool(name="consts", bufs=1))
    p_x32 = ctx.enter_context(tc.tile_pool(name="x32", bufs=2))
    p_xbf = ctx.enter_context(tc.tile_pool(name="xbf", bufs=2))
    p_t3 = ctx.enter_context(tc.tile_pool(name="t3", bufs=2))
    p_t2t = ctx.enter_context(tc.tile_pool(name="t2t", bufs=1))
    p_t6n = ctx.enter_context(tc.tile_pool(name="t6n", bufs=2))
    p_t6t = ctx.enter_context(tc.tile_pool(name="t6t", bufs=2))
    p_t7sb = ctx.enter_context(tc.tile_pool(name="t7sb", bufs=2))
    p_fsb = ctx.enter_context(tc.tile_pool(name="fsb", bufs=3))
    p_out = ctx.enter_context(tc.tile_pool(name="outp", bufs=2))
    ps_xt = ctx.enter_context(tc.tile_pool(name="psxt", bufs=2, space="PSUM"))
    ps_t7 = ctx.enter_context(tc.tile_pool(name="pst7", bufs=1, space="PSUM"))
    ps_t10 = ctx.enter_context(tc.tile_pool(name="pst10", bufs=2, space="PSUM"))
    ps_t8 = ctx.enter_context(tc.tile_pool(name="pst8", bufs=1, space="PSUM"))

    p2wt_t = consts.tile([PCH, NCH * 3 * C], BF)
    nc.sync.dma_start(p2wt_t[:], p2wt)
    w2t_t = consts.tile([C, C], BF)
    nc.sync.dma_start(w2t_t[:], w2t)
    id_t = consts.tile([C, C], BF)
    nc.sync.dma_start(id_t[:], ident)

    def r3(ap, k):
        return ap.rearrange("p (j k) -> p j k", k=k)

    # t2^T buffer is persistent; its two boundary blocks (j=0,d=0) and
    # (j=27,d=2) are zero forever (the shifted x there is zero padding).
    t2t = p_t2t.tile([PCH, NCH * 3 * C], BF, tag="t2t")
    nc.gpsimd.memset(t2t[:, 0:C], 0.0)
    nc.gpsimd.memset(t2t[:, (NCH * 3 - 1) * C : NCH * 3 * C], 0.0)
    t2r = t2t[:].rearrange("p (j x) -> p j x", x=3 * C)
    pwr = p2wt_t[:].rearrange("p (j x) -> p j x", x=3 * C)

    for s in range(NS):
        x32 = p_x32.tile([C, HW], F32, tag="x32")
        nc.sync.dma_start(x32[:], xs[s])

        xbf = p_xbf.tile([C, HW], BF, tag="xbf")
        nc.gpsimd.tensor_copy(xbf[:], x32[:])  # fp32 -> bf16 cast on Pool

        xbf3 = xbf[:].rearrange("c (h w) -> c h w", h=H)
        # t3_0 (cols shifted +2 with zero pad) on ACT, t3_2 on DVE
        t30 = p_t3.tile([C, H, W], BF, tag="t30")
        nc.gpsimd.memset(t30[:, :, 0:2], 0.0)
        nc.scalar.copy(t30[:, :, 2:W], xbf3[:, :, 0 : W - 2])
        t32 = p_t3.tile([C, H, W], BF, tag="t32")
        nc.gpsimd.memset(t32[:, :, W - 2 : W], 0.0)
        nc.vector.tensor_copy(t32[:, :, 0 : W - 2], xbf3[:, :, 2:W])
        t30f = t30[:].rearrange("c h w -> c (h w)")
        t32f = t32[:].rearrange("c h w -> c (h w)")

        # --- transpose x into PSUM, two half-groups of 15 chunks ---
        # group 0 holds real chunks 0..14 at locals 0..14
        # group 1 holds real chunks 13..27 at locals 0..14
        xt0 = ps_xt.tile([PCH, GB * C], BF, tag="xt")
        xt1 = ps_xt.tile([PCH, GB * C], BF, tag="xt")
        for r in range(0, 15):
            nc.tensor.transpose(
                xt0[:, r * C : (r + 1) * C],
                xbf[:, r * PCH : (r + 1) * PCH],
                id_t[:],
            )
        for r in range(13, 28):
            nc.tensor.transpose(
                xt1[:, (r - 13) * C : (r - 12) * C],
                xbf[:, r * PCH : (r + 1) * PCH],
                id_t[:],
            )

        # --- t2^T = p2w^T * x^T(shifted) ---  (112 x 28*384, free = [j][d][c1])
        # output block (j, d) multiplies x^T real chunk (j + d - 1)
        for d in range(3):
            j0, j1 = max(0, 1 - d), min(HCH, HCH)  # g0 j range
            lo = j0 + d - 1
            nc.vector.tensor_tensor(
                out=t2r[:, j0:HCH, d * C : (d + 1) * C],
                in0=pwr[:, j0:HCH, d * C : (d + 1) * C],
                in1=r3(xt0[:, lo * C : (lo + HCH - j0) * C], C),
                op=mult,
            )
        for d in range(3):
            j0, j1 = HCH, NCH if d < 2 else NCH - 1  # g1 j range
            lo = j0 + d - 1 - 13
            nc.vector.tensor_tensor(
                out=t2r[:, j0:j1, d * C : (d + 1) * C],
                in0=pwr[:, j0:j1, d * C : (d + 1) * C],
                in1=r3(xt1[:, lo * C : (lo + j1 - j0) * C], C),
                op=mult,
            )

        # --- t6 = x + roll(x, 1 row) built in natural layout (free shifts) ---
        t6n = p_t6n.tile([C, HW], BF, tag="t6n")
        nc.vector.tensor_tensor(
            out=t6n[:, W:HW], in0=xbf[:, W:HW], in1=xbf[:, 0 : HW - W], op=addop
        )
        nc.vector.tensor_tensor(
            out=t6n[:, 0:W], in0=xbf[:, 0:W], in1=xbf[:, HW - W : HW], op=addop
        )
        # transpose t6n -> psum (two half groups), then batch-copy to SBUF
        t6t = p_t6t.tile([PCH, NCH * C], BF, tag="t6t")
        for g in range(2):
            yg = ps_xt.tile([PCH, GB * C], BF, tag="xt")
            for r in range(HCH):
                j = g * HCH + r
                nc.tensor.transpose(
                    yg[:, r * C : (r + 1) * C],
                    t6n[:, j * PCH : (j + 1) * PCH],
                    id_t[:],
                )
            nc.vector.tensor_copy(
                t6t[:, g * HCH * C : (g + 1) * HCH * C], yg[:, 0 : HCH * C]
            )

        # --- t7^T accumulation: (c x (d,c1)) ---
        t7ps = ps_t7.tile([C, 3 * C], F32, tag="t7")
        for j in range(NCH):
            nc.tensor.matmul(
                t7ps[:],
                t6t[:, j * C : (j + 1) * C],
                t2t[:, j * 3 * C : (j + 1) * 3 * C],
                start=(j == 0),
                stop=(j == NCH - 1),
            )
        t7T_sb = p_t7sb.tile([C, 3 * C], BF, tag="t7T")
        nc.vector.tensor_copy(t7T_sb[:], t7ps[:])
        t7dps = ps_t7.tile([C, 3 * C], BF, tag="t7")
        for d in range(3):
            nc.tensor.transpose(
                t7dps[:, d * C : (d + 1) * C],
                t7T_sb[:, d * C : (d + 1) * C],
                id_t[:],
            )
        t7d_sb = p_t7sb.tile([C, 3 * C], BF, tag="t7d")
        nc.vector.tensor_copy(t7d_sb[:], t7dps[:])

        # --- output stage ---
        outsb = p_out.tile([C, HW], F32, tag="outp")
        t3list = (t30f, xbf[:], t32f)
        for f in range(NF):
            lo = f * FCH
            t8ps = ps_t8.tile([C, FCH], F32, tag="t8")
            if f == 0:
                nc.tensor.matmul(
                    t8ps[:, 0:W], w2t_t[:], xbf[:, HW - W : HW],
                    start=True, stop=False,
                )
                nc.tensor.matmul(
                    t8ps[:, W:FCH], w2t_t[:], xbf[:, 0 : FCH - W],
                    start=False, stop=True,
                )
            else:
                nc.tensor.matmul(
                    t8ps[:], w2t_t[:], xbf[:, lo - W : lo + FCH - W],
                    start=True, stop=True,
                )
            t10ps = ps_t10.tile([C, FCH], F32, tag="t10")
            for d in range(3):
                nc.tensor.matmul(
                    t10ps[:],
                    t7d_sb[:, d * C : (d + 1) * C],
                    t3list[d][:, lo : lo + FCH],
                    start=(d == 0),
                    stop=(d == 2),
                )
            t8sb = p_fsb.tile([C, FCH], F32, tag="t8sb")
            nc.scalar.copy(t8sb[:], t8ps[:])
            t10sb = p_fsb.tile([C, FCH], F32, tag="t10sb")
            nc.scalar.copy(t10sb[:], t10ps[:])
            t9sb = p_fsb.tile([C, FCH], F32, tag="t9sb")
            nc.vector.tensor_tensor(
                out=t9sb[:], in0=t8sb[:], in1=x32[:, lo : lo + FCH], op=mult
            )
            nc.vector.tensor_tensor(
                out=outsb[:, lo : lo + FCH], in0=t9sb[:], in1=t10sb[:], op=addop
            )
        nc.sync.dma_start(out[s], outsb[:])


def build():
    if "nc" in _CACHE:
        return _CACHE["nc"]
    from contextlib import ExitStack

    import concourse.bass as bass
    import concourse.tile as tile
    from concourse import bacc, mybir

    BF = mybir.dt.bfloat16
    F32 = mybir.dt.float32
    nc = bacc.Bacc("TRN2", target_bir_lowering=False, debug=False)
    xs = nc.dram_tensor("xs", [NS, C, HW], F32, kind="ExternalInput").ap()
    p2wt = nc.dram_tensor("p2wt", [PCH, NCH * 3 * C], BF, kind="ExternalInput").ap()
    w2t = nc.dram_tensor("w2t", [C, C], BF, kind="ExternalInput").ap()
    ident = nc.dram_tensor("ident", [C, C], BF, kind="ExternalInput").ap()
    out = nc.dram_tensor("out", [NS, C, HW], F32, kind="ExternalOutput").ap()

    with tile.TileContext(nc) as tc:
        with __import__("contextlib").ExitStack() as ctx:
            _body(tc, bass, mybir, xs, p2wt, w2t, ident, out, BF, F32, ctx)
    nc.compile()
    _CACHE["nc"] = nc
    return nc


def host_inputs(x, p2w, p5w, conv_w):
    """Shard + prep per-core input maps from full inputs."""
    x = np.ascontiguousarray(np.asarray(x, dtype=np.float32).reshape(N, C, HW))
    p2w_ = (np.asarray(p2w, dtype=np.float32)[0] * SCALE).reshape(C, 3, HW)
    a = p2w_.transpose(2, 1, 0)                      # (p, d, c1)
    a = a.reshape(NCH, PCH, 3, C)                    # (j, pl, d, c1)
    a = np.ascontiguousarray(a.transpose(1, 0, 2, 3)).reshape(PCH, NCH * 3 * C)
    p2wt = a.astype(BF16NP)
    p5 = np.asarray(p5w, dtype=np.float32).reshape(C)
    cw = np.asarray(conv_w, dtype=np.float32)        # (C//G, C)
    W2 = cw[np.arange(C) % (C // G)] * p5[None, :]   # (c, c')
    w2t = np.ascontiguousarray(W2.T).astype(BF16NP)  # (c', c)
    ident = np.eye(C, dtype=BF16NP)
    in_maps = [
        {
            "xs": np.ascontiguousarray(x[i * NS : (i + 1) * NS]),
            "p2wt": p2wt,
            "w2t": w2t,
            "ident": ident,
        }
        for i in range(NCORES)
    ]
    return in_maps


def _get_runner():
    """Build (once) a persistent jitted shard_map executable over 8 cores."""
    if "runner" in _CACHE:
        return _CACHE["runner"]
    import jax
    from jax.sharding import Mesh, PartitionSpec
    from jax.experimental.shard_map import shard_map
    from concourse import bass2jax, mybir

    nc = build()
    bass2jax.install_neuronx_cc_hook()

    partition_name = nc.partition_id_tensor.name if nc.partition_id_tensor else None
    in_names, out_names, out_avals, zero_outs = [], [], [], []
    for alloc in nc.m.functions[0].allocations:
        if not isinstance(alloc, mybir.MemoryLocationSet):
            continue
        name = alloc.memorylocations[0].name
        if alloc.kind == "ExternalInput":
            if name != partition_name:
                in_names.append(name)
        elif alloc.kind == "ExternalOutput":
            shape = tuple(alloc.tensor_shape)
            dtype = mybir.dt.np(alloc.dtype)
            out_avals.append(jax.core.ShapedArray(shape, dtype))
            zero_outs.append(np.zeros(shape, dtype))
            out_names.append(name)
    n_params = len(in_names)
    n_outs = len(out_avals)
    all_in_names = list(in_names) + list(out_names)
    if partition_name is not None:
        all_in_names.append(partition_name)
    donate = tuple(range(n_params, n_params + n_outs))

    def _body(*args):
        operands = list(args)
        if partition_name is not None:
            operands.append(bass2jax.partition_id_tensor())
        outs = bass2jax._bass_exec_p.bind(
            *operands,
            out_avals=tuple(out_avals),
            in_names=tuple(all_in_names),
            out_names=tuple(out_names),
            lowering_input_output_aliases=(),
            sim_require_finite=True,
            sim_require_nnan=True,
            nc=nc,
        )
        return tuple(outs)

    devices = jax.devices()[:NCORES]
    mesh = Mesh(np.asarray(devices), ("core",))
    in_specs = (PartitionSpec("core"),) * (n_params + n_outs)
    out_specs = (PartitionSpec("core"),) * n_outs
    sharded = jax.jit(
        shard_map(
            _body, mesh=mesh, in_specs=in_specs, out_specs=out_specs, check_rep=False
        ),
        donate_argnums=donate,
        keep_unused=True,
    )
    runner = {
        "fn": sharded,
        "in_names": in_names,
        "out_names": out_names,
        "out_avals": out_avals,
        "mesh": mesh,
        "n_params": n_params,
    }
    _CACHE["runner"] = runner
    return runner


def _concat_inputs(runner, in_maps):
    return [
        np.concatenate([np.asarray(m[name]) for m in in_maps], axis=0)
        for name in runner["in_names"]
    ]


def _zero_bufs(runner):
    return [
        np.zeros((NCORES * a.shape[0], *a.shape[1:]), a.dtype)
        for a in runner["out_avals"]
    ]


def run_fast(in_maps):
    """Execute via the cached jitted executable; returns list of per-core dicts."""
    runner = _get_runner()
    out_arrs = runner["fn"](*_concat_inputs(runner, in_maps), *_zero_bufs(runner))
    res = []
    for c in range(NCORES):
        res.append(
            {
                name: np.asarray(out_arrs[i]).reshape(
                    NCORES, *runner["out_avals"][i].shape
                )[c]
                for i, name in enumerate(runner["out_names"])
            }
        )
    return res


def run(in_maps, trace=False, **kw):
    from concourse.bass_utils import run_bass_kernel_spmd

    nc = build()
    return run_bass_kernel_spmd(nc, in_maps, list(range(NCORES)), trace=trace, **kw)


def bench(in_maps, iters=30):
    """Pipelined timing of the device executable. Returns sec/iter."""
    import time

    import jax
    from jax.sharding import NamedSharding, PartitionSpec

    runner = _get_runner()
    sh = NamedSharding(runner["mesh"], PartitionSpec("core"))
    dev_in = [jax.device_put(a, sh) for a in _concat_inputs(runner, in_maps)]
    # donated output buffers are consumed per call: pre-stage iters copies
    zsets = [
        [jax.device_put(z, sh) for z in _zero_bufs(runner)] for _ in range(iters + 2)
    ]
    for z in zsets[0]:
        z.block_until_ready()
    # warmup
    out = runner["fn"](*dev_in, *zsets[-1])
    jax.block_until_ready(out)
    out = runner["fn"](*dev_in, *zsets[-2])
    jax.block_until_ready(out)
    t0 = time.perf_counter()
    outs = []
    for k in range(iters):
        outs.append(runner["fn"](*dev_in, *zsets[k]))
    jax.block_until_ready(outs)
    dt = (time.perf_counter() - t0) / iters
    return dt


def kernel(x, p2w, p5w, conv_w):
    in_maps = host_inputs(x, p2w, p5w, conv_w)
    res = run_fast(in_maps)
    outs = [np.asarray(res[i]["out"]) for i in range(NCORES)]
    return np.concatenate(outs, axis=0).reshape(N, C, H, W).astype(np.float32)
